# revision 1
# baseline (speedup 1.0000x reference)
"""CircleLoss kernel for Trainium2 (Bass/Tile), SPMD across 8 NeuronCores.

Math (s=32, m=0.25, B=8192, D=128):
    u = cos-sim matrix (row-normalized feats @ feats.T)
    p-side: since u <= 1 < 1+m, relu(1+m-u) = 1+m-u always, and
        expo_p = -s*(1+m-u)*(u-(1-m)) = s*(1-u)^2 - s*m^2 = s*(1-u)^2 - 2
    n-side: r = relu(u+m), expo_n = s*r*(u-m) = s*(r-m)^2 - s*m^2  (exact
        also at r=0: exp(0)=1 matches reference's exp(0)=1? no - reference
        gives exp(0)=1 only for masked-in entries; masking handled below)
    Masking: one bf16 mask mn = 4*(label_i != label_j), built by a DVE
    is_equal against partition-replicated labels, is folded additively into
    both exp arguments (p: t = w - mn with bias -2; n: t2 = w2 + mn with
    bias -130 = -(4*s + 2)) so masked-out entries underflow to 0.
    Diagonal: self-sim u_ii ~= 1 and same_ii=1, so the unmasked p-sum gains
    exactly exp(-2) per row; subtracted as a constant on the host.

Implementation: per core, 8192x128 feats are normalized + PE-transposed into
fT [128, 8192] f32r once; the [1024, 8192] slab is processed in 32 chunks of
[128 rows x 2048 cols]: 4 f32r matmuls fill a 4-bank PSUM tile, a single
PSUM read converts u to bf16 (alternating ACT/DVE), and the rest of the
chain runs as cheap bf16 DVE 2x-mode ops (p-square alternates ACT Square /
DVE+Pool multiply), with ACT Exp+row-accumulate producing per-row partial
sums. Emission is software-pipelined in 3 skewed stages so no engine
head-of-line blocks. Measured ~220-275us/core on HW (~55us prep + ~165-220us
main), rel err vs reference ~5e-4 (bf16 intermediates).

Sharding: core c owns rows [c*1024, (c+1)*1024). Each core loads the full
feats, normalizes+transposes locally (cheaper than any collective), computes
its [1024, 8192] slab, and returns per-row partial sums P,N packed [128,16].
Host finishes: P -= e^-2, loss = mean(log1p(P*N)).
"""

import os
import numpy as np
from contextlib import ExitStack

import concourse.bass as bass
import concourse.bacc as bacc
import concourse.tile as tile
import concourse.mybir as mybir
from concourse.bass_utils import run_bass_kernel_spmd

B, D, NCORES = 8192, 128, 8
BL = B // NCORES          # 1024 rows per core
S, M = 32.0, 0.25
MASKV = 4.0               # mask magnitude: exp arg shifts by -S*MASKV = -128
NCH = 512                 # similarity n-chunk width (1 PSUM bank of f32)
N_NCK = B // NCH          # 16
N_MC = BL // 128          # 8 row-chunks of 128 per core
F32 = mybir.dt.float32
F32R = mybir.dt.float32r
BF16 = mybir.dt.bfloat16
AF = mybir.ActivationFunctionType
ALU = mybir.AluOpType

_NC_CACHE = None
LAST_RESULTS = None       # BassKernelResults of the most recent run


def _register_consts(nc, values):
    # Arbitrary activation biases need a const AP; mimic Bass.__init__'s
    # register_const_ap (memset before Tile's instruction stream + barrier).
    for v in values:
        key = (F32, float(v))
        if key in nc.const_aps.aps:
            continue
        t = nc.alloc_sbuf_tensor(f"const-f32-{v}", [128, 1], F32)
        nc.gpsimd.memset(t.ap(), float(v))
        nc.const_aps.aps[key] = t.ap()
    nc.all_engine_barrier()


def _build_nc(loops=1, prep=1):
    nc = bacc.Bacc(
        "TRN2", target_bir_lowering=False, debug=False, num_devices=NCORES
    )
    _register_consts(nc, [-2.0, -130.0])
    feats = nc.dram_tensor("feats", [B, D], F32, kind="ExternalInput").ap()
    floc = nc.dram_tensor("floc", [BL, D], F32, kind="ExternalInput").ap()
    lab_all = nc.dram_tensor("lab_all", [1, B], BF16, kind="ExternalInput").ap()
    lab_loc = nc.dram_tensor("lab_loc", [128, N_MC], F32, kind="ExternalInput").ap()
    ident = nc.dram_tensor("ident", [128, 128], F32, kind="ExternalInput").ap()
    out = nc.dram_tensor("out", [128, 2 * N_MC], F32, kind="ExternalOutput").ap()

    with tile.TileContext(nc) as tc, ExitStack() as ctx:
        persist = ctx.enter_context(tc.tile_pool(name="persist", bufs=1))
        ft_pool = ctx.enter_context(tc.tile_pool(name="ft", bufs=3))
        sm_pool = ctx.enter_context(tc.tile_pool(name="sm", bufs=4))

        fT = persist.tile([128, B], F32R, name="fT")
        fTloc = persist.tile([128, BL], F32R, name="fTloc")
        labR = persist.tile([128, B], BF16, name="labR")
        labL = persist.tile([128, N_MC], F32, name="labL")
        idn = persist.tile([128, 128], F32, name="idn")
        stats = persist.tile([128, 2 * N_MC], F32, name="stats")

        nc.sync.dma_start(out=idn[:], in_=ident)
        nc.sync.dma_start(out=labR[:], in_=lab_all.to_broadcast((128, B)))
        nc.sync.dma_start(out=labL[:], in_=lab_loc)

        def norm_transpose(tp_pool, src_dram, n_rows, dst):
            """dst[:, i] = src[i, :] / ||src[i, :]|| for i in range(n_rows)."""
            for t in range(n_rows // 128):
                ftile = ft_pool.tile([128, D], F32, tag="ftile")
                nc.sync.dma_start(
                    out=ftile[:], in_=src_dram[t * 128 : (t + 1) * 128, :]
                )
                sq = ft_pool.tile([128, D], F32, tag="sq")
                nc.gpsimd.tensor_mul(sq[:], ftile[:], ftile[:])
                ssq = sm_pool.tile([128, 1], F32, tag="ssq")
                nc.vector.tensor_reduce(
                    ssq[:], sq[:], axis=mybir.AxisListType.X, op=ALU.add
                )
                nrm = sm_pool.tile([128, 1], F32, tag="nrm")
                nc.scalar.activation(nrm[:], ssq[:], AF.Sqrt)
                inv = sm_pool.tile([128, 1], F32, tag="inv")
                nc.vector.reciprocal(inv[:], nrm[:])
                fnorm = ft_pool.tile([128, D], F32, tag="fnorm")
                nc.vector.tensor_scalar_mul(fnorm[:], ftile[:], inv[:])
                pt = tp_pool.tile([128, 128], F32, tag="pt")
                nc.tensor.transpose(pt[:], fnorm[:], idn[:])
                if t % 2 == 0:
                    nc.vector.tensor_copy(dst[:, t * 128 : (t + 1) * 128], pt[:])
                else:
                    nc.scalar.copy(dst[:, t * 128 : (t + 1) * 128], pt[:])

        if prep:
            with tc.tile_pool(name="tp", bufs=2, space="PSUM") as tp_pool:
                for _prep_rep in range(prep):
                    norm_transpose(tp_pool, floc, BL, fTloc)
                    norm_transpose(tp_pool, feats, B, fT)

        # Main loop v3. EW=2048-wide elementwise (4 matmul quarters per
        # 4-bank PSUM tile). One bf16 mask mn = 4*(label_i != label_j) kills
        # both sides: p-arg = S*(w - mn) - 2 (w=(1-u)^2), n-arg =
        # S*(w2 + mn) - 130 (w2 = max(u,-m)^2; keep needs mn=4: -130+128=-2).
        # bf16 intermediates give DVE its 2x mode on all SBUF tensor-tensor
        # ops; the p-square alternates ACT Square / DVE STT (u^2-2u, bias
        # +30) 3:1 to balance engine load. Pool builds the mask.
        EW = 4 * NCH              # 2048
        N_EW = B // EW            # 4
        ps_pool = ctx.enter_context(tc.tile_pool(name="ps", bufs=2, space="PSUM"))
        el_pool = ctx.enter_context(tc.tile_pool(name="el", bufs=int(os.environ.get("ELBUFS", "4"))))
        ex_pool = ctx.enter_context(tc.tile_pool(name="ex", bufs=int(os.environ.get("EXBUFS", "2"))))
        st_pool = ctx.enter_context(tc.tile_pool(name="st", bufs=2))

        chunks = [(mc, ew) for mc in range(N_MC) for ew in range(N_EW)]
        T = len(chunks)
        live = {}
        pstats, nstats = {}, {}
        pairt = {}

        def s0(c):
            mc, ew = chunks[c]
            if ew == 0:
                pstats[mc] = st_pool.tile([128, N_EW // 2], F32, tag="pstat", name="pstat")
                nstats[mc] = st_pool.tile([128, N_EW // 2], F32, tag="nstat", name="nstat")
            lhs_f = fTloc[:, mc * 128 : (mc + 1) * 128]
            ps = ps_pool.tile([128, EW], F32, tag="ps")
            for h in range(4):
                nsl = slice(ew * EW + h * NCH, ew * EW + (h + 1) * NCH)
                hsl = slice(h * NCH, (h + 1) * NCH)
                nc.tensor.matmul(
                    ps[:, hsl], lhs_f, fT[:, nsl], start=True, stop=True
                )
            esl = slice(ew * EW, (ew + 1) * EW)
            mn = el_pool.tile([128, EW], BF16, tag="mn")
            nc.vector.tensor_scalar(
                mn[:], labR[:, esl], labL[:, mc : mc + 1], MASKV,
                op0=ALU.not_equal, op1=ALU.mult,
            )
            # single PSUM read: u16 = bf16(u); everything downstream runs
            # in DVE 2x mode. Copy engine alternates ACT/DVE to balance.
            u16 = el_pool.tile([128, EW], BF16, tag="u16")
            _CM = int(os.environ.get("COPYMOD", "4"))
            if c % _CM != 0:
                nc.scalar.copy(u16[:], ps[:])
            else:
                nc.vector.tensor_copy(u16[:], ps[:])
            live[c] = [mn, u16]

        def s1(c):
            mn, u16 = live[c]
            mc, ew = chunks[c]
            if ew % 2 == 0:
                pairt[mc, ew // 2] = (
                    el_pool.tile([128, 2 * EW], BF16, tag="t", name="t", bufs=2),
                    el_pool.tile([128, 2 * EW], BF16, tag="t2", name="t2", bufs=2),
                )
            tp, tp2 = pairt[mc, ew // 2]
            hsl = slice((ew % 2) * EW, (ew % 2 + 1) * EW)
            # p: w = (1-u)^2;  t = w - mn   (exp bias -2)
            w = el_pool.tile([128, EW], BF16, tag="w", bufs=2)
            if c % 2 == 0:
                nc.scalar.activation(w[:], u16[:], AF.Square, bias=1.0, scale=-1.0)
            else:
                a = el_pool.tile([128, EW], BF16, tag="a", bufs=2)
                nc.vector.tensor_scalar(
                    a[:], u16[:], -1.0, 1.0, op0=ALU.mult, op1=ALU.add
                )
                nc.gpsimd.tensor_mul(w[:], a[:], a[:])
            nc.vector.tensor_sub(tp[:, hsl], w[:], mn[:])
            # n: r = max(u,-m); w2 = r^2; t2 = w2 + mn  (exp bias -130)
            r = el_pool.tile([128, EW], BF16, tag="r", bufs=2)
            nc.vector.tensor_scalar(r[:], u16[:], -M, None, op0=ALU.max)
            w2 = el_pool.tile([128, EW], BF16, tag="w2", bufs=2)
            nc.vector.tensor_mul(w2[:], r[:], r[:])
            nc.vector.tensor_add(tp2[:, hsl], w2[:], mn[:])
            live[c] = None

        def s2(c):
            mc, ew = chunks[c]
            live.pop(c)
            if ew % 2 == 0:
                return
            t, t2 = pairt.pop((mc, ew // 2))
            pexp = ex_pool.tile([128, 2 * EW], F32, tag="escr")
            nc.scalar.activation(
                pexp[:], t[:], AF.Exp, bias=-2.0, scale=S,
                accum_out=pstats[mc][:, ew // 2 : ew // 2 + 1],
            )
            nexp = ex_pool.tile([128, 2 * EW], F32, tag="escr")
            nc.scalar.activation(
                nexp[:], t2[:], AF.Exp, bias=-130.0, scale=S,
                accum_out=nstats[mc][:, ew // 2 : ew // 2 + 1],
            )
            if ew == N_EW - 1:
                nc.vector.tensor_reduce(
                    stats[:, mc : mc + 1], pstats[mc][:],
                    axis=mybir.AxisListType.X, op=ALU.add,
                )
                nc.vector.tensor_reduce(
                    stats[:, N_MC + mc : N_MC + mc + 1], nstats[mc][:],
                    axis=mybir.AxisListType.X, op=ALU.add,
                )

        if loops == 0:
            nc.gpsimd.memset(stats[:], 0.0)
        import os as _os
        _SK = int(_os.environ.get("SKEW", "3"))
        for rep in range(loops):
            live.clear(); pstats.clear(); nstats.clear(); pairt.clear()
            for c in range(T + _SK):
                if c < T:
                    s0(c)
                if 1 <= c and c - 1 < T:
                    s1(c - 1)
                if _SK <= c and c - _SK < T:
                    s2(c - _SK)
        nc.sync.dma_start(out=out, in_=stats[:])
    nc.compile()
    return nc


def kernel(feats, labels):
    global _NC_CACHE, LAST_RESULTS
    feats = np.ascontiguousarray(np.asarray(feats), dtype=np.float32)
    labels = np.asarray(labels).reshape(-1)
    import ml_dtypes
    lab_bf = labels.astype(ml_dtypes.bfloat16).reshape(1, -1)
    ident = np.eye(128, dtype=np.float32)

    if _NC_CACHE is None:
        _NC_CACHE = _build_nc()
    nc = _NC_CACHE

    in_maps = []
    for c in range(NCORES):
        sl = slice(c * BL, (c + 1) * BL)
        in_maps.append({
            "feats": feats,
            "floc": np.ascontiguousarray(feats[sl]),
            "lab_all": lab_bf,
            "lab_loc": np.ascontiguousarray(
                labels[sl].reshape(N_MC, 128).T.astype(np.float32)
            ),
            "ident": ident,
        })
    res = run_bass_kernel_spmd(
        nc, in_maps, list(range(NCORES)),
        trace=bool(os.environ.get("KERNEL_TRACE")),
    )
    LAST_RESULTS = res

    P_parts, N_parts = [], []
    for c in range(NCORES):
        st = res.results[c]["out"]            # [128, 16]
        P_parts.append(st[:, :N_MC].T.reshape(-1))    # row g=mc*128+p
        N_parts.append(st[:, N_MC:].T.reshape(-1))
    P = np.concatenate(P_parts) - np.float32(np.exp(-2.0))
    N = np.concatenate(N_parts)
    loss_rows = np.log1p((P.astype(np.float32) * N.astype(np.float32)))
    return np.float32(np.mean(loss_rows))



# revision 24
# speedup vs baseline: 1.7274x; 1.7274x over previous
"""CircleLoss kernel for Trainium2 (Bass/Tile), SPMD across 8 NeuronCores.

v2: label-sorted + per-core rotated layout.

Math (s=32, m=0.25, B=8192, D=128), u = cosine sim:
    p-side: expo_p = s*(1-u)^2 - 2           (relu never clips: u <= 1)
    n-side: expo_n = s*max(u,-m)^2 - 2       (exact, incl. clamp at -m)
    loss = mean(log1p((P - e^-2) * N)), P/N per-row sums of exp(expo).

Layout trick: rows are sorted by label on the host. Each core's input is
the sorted row set ROTATED so its own 1024 rows sit at columns
[960, 1984) of its local fT. All same-label columns for local row-chunk
mc then provably lie in a fixed window [mc*128+WOFF0, +W) (host verifies
group sizes; W=1536 or 2048). The SPMD program is identical across
cores; all data-dependence lives in the inputs (rotated feats + labels).

Per row-chunk mc (128 rows):
  - n-side: 4 PSUM groups of 2048 cols over all 8192 cols. Columns
    outside the window are guaranteed different-label -> no mask:
    r = max(u,-m) (Pool/DVE/ACT), w2 = r*r into bf16 staging. Window
    columns get a mask mn2 = -4*(lab_i==lab_j) folded additively
    (exp arg shifts by -128 -> 0). Two Exp+accum per row [128,4096].
  - p-side: 1 extra PSUM tile [128, W] over the window columns only:
    w = (1-u)^2 via ACT Square from PSUM, tp = w - mn (mn=4*(neq)),
    one Exp+accum [128, W]. Diagonal contributes exp(-2), subtracted
    on the host.

Output per core: [128, 24] f32: cols 0..7 = p row-sums per mc,
cols 8+2mc, 9+2mc = n partial row-sums. Host: mean(log1p((P-e^-2)*N)).
"""

import os
import numpy as np
from contextlib import ExitStack

import concourse.bass as bass
import concourse.bacc as bacc
import concourse.tile as tile
import concourse.mybir as mybir
from concourse.bass_utils import run_bass_kernel_spmd

B, D, NCORES = 8192, 128, 8
BL = B // NCORES          # 1024 rows per core
N_MC = BL // 128          # 8 row-chunks per core
S, M = 32.0, 0.25
MASKV = 4.0               # exp-arg shift: -S*4 = -128
ROT = 960                 # local rows sit at rotated cols [960, 1984)
GW = 2048                 # PSUM group width (4 banks)
PAIR = 4096               # n-exp width (2 groups)
F32 = mybir.dt.float32
F32R = mybir.dt.float32r
BF16 = mybir.dt.bfloat16
AF = mybir.ActivationFunctionType
ALU = mybir.AluOpType

_NC_CACHE = {}
LAST_RESULTS = None


def _register_consts(nc, values):
    for v in values:
        key = (F32, float(v))
        if key in nc.const_aps.aps:
            continue
        t = nc.alloc_sbuf_tensor(f"const-f32-{v}", [128, 1], F32)
        nc.gpsimd.memset(t.ap(), float(v))
        nc.const_aps.aps[key] = t.ap()
    nc.all_engine_barrier()


def _build_nc(loops=1, prep=1, W=1536):
    WOFF0 = ROT - (W - 128) // 2      # 256 for W=1536, 0 for W=2048
    nc = bacc.Bacc(
        "TRN2", target_bir_lowering=False, debug=False, num_devices=NCORES
    )
    _register_consts(nc, [-2.0, 0.25, -0.25])
    feats = nc.dram_tensor("feats", [B, D], F32, kind="ExternalInput").ap()
    lab_all = nc.dram_tensor("lab_all", [1, B], BF16, kind="ExternalInput").ap()
    lab_loc = nc.dram_tensor("lab_loc", [128, N_MC], F32, kind="ExternalInput").ap()
    ident = nc.dram_tensor("ident", [128, 128], F32, kind="ExternalInput").ap()
    out = nc.dram_tensor("out", [128, 3 * N_MC], F32, kind="ExternalOutput").ap()

    # engine-load tracker for greedy assignment (ns, coarse cost model)
    load = {"ACT": 0.0, "DVE": 0.0, "POOL": 0.0}

    plan_hist = {}

    def pick(variants):
        """variants: (key, {eng: cost}) list; min-max with full-vector
        lexicographic tie-break so ties don't fall to declaration order."""
        best, bestv = None, None
        for key, costs in variants:
            vec = sorted(
                (load[e] + costs.get(e, 0.0) for e in load), reverse=True
            )
            if bestv is None or vec < bestv:
                best, bestv = key, vec
        _, costs = next(v for v in variants if v[0] == best)
        for e, c in costs.items():
            load[e] += c
        plan_hist[best] = plan_hist.get(best, 0) + 1
        return best

    with tile.TileContext(nc) as tc, ExitStack() as ctx:
        persist = ctx.enter_context(tc.tile_pool(name="persist", bufs=1))
        ft_pool = ctx.enter_context(tc.tile_pool(name="ft", bufs=3))
        sm_pool = ctx.enter_context(tc.tile_pool(name="sm", bufs=4))

        fT = persist.tile([128, B], F32R, name="fT")
        labR = persist.tile([128, B], BF16, name="labR")
        labL = persist.tile([128, N_MC], F32, name="labL")
        idn = persist.tile([128, 128], F32, name="idn")
        stats = persist.tile([128, 3 * N_MC], F32, name="stats")

        nc.sync.dma_start(out=idn[:], in_=ident)
        nc.sync.dma_start(out=labL[:], in_=lab_loc)

        def norm_transpose_batch(tp_pool, b):
            """Normalize+transpose rows [b*1024, (b+1)*1024) into fT cols."""
            load["POOL"] += 860                    # sq
            load["DVE"] += 1127 + 100 + 4 * 127 + 4 * 258
            load["ACT"] += 4 * 292                 # fnorm act-copies
            load["ACT"] += 192 + 4 * 292          # sqrt + copies
            fb = ft_pool.tile([128, 8, 128], F32, tag="fb")
            for t in range(8):
                r0 = b * 1024 + t * 128
                nc.sync.dma_start(out=fb[:, t, :], in_=feats[r0 : r0 + 128, :])
            sq = ft_pool.tile([128, 8, 128], F32, tag="sq")
            nc.gpsimd.tensor_mul(sq[:], fb[:], fb[:])
            ssq = sm_pool.tile([128, 8, 1], F32, tag="ssq")
            nc.vector.tensor_reduce(
                ssq[:], sq[:], axis=mybir.AxisListType.X, op=ALU.add
            )
            nrm = sm_pool.tile([128, 8, 1], F32, tag="nrm")
            nc.scalar.activation(nrm[:], ssq[:], AF.Sqrt)
            inv = sm_pool.tile([128, 8, 1], F32, tag="inv")
            nc.vector.reciprocal(inv[:], nrm[:])
            for t in range(8):
                fn = ft_pool.tile([128, 128], F32, tag="fn", bufs=4)
                if t % 2 == 0:
                    nc.vector.tensor_scalar_mul(fn[:], fb[:, t, :], inv[:, t, :])
                else:
                    nc.scalar.activation(
                        fn[:], fb[:, t, :], AF.Copy, scale=inv[:, t, :]
                    )
                pt = tp_pool.tile([128, 128], F32, tag="pt")
                nc.tensor.transpose(pt[:], fn[:], idn[:])
                c0 = (b * 8 + t) * 128
                if t % 2 == 0:
                    nc.vector.tensor_copy(fT[:, c0 : c0 + 128], pt[:])
                else:
                    nc.scalar.copy(fT[:, c0 : c0 + 128], pt[:])

        tp_pool = ctx.enter_context(tc.tile_pool(name="tp", bufs=2, space="PSUM"))
        if prep:
            for _rep in range(prep):
                for b in range(8):
                    norm_transpose_batch(tp_pool, b)
                    if b == 0 and _rep == 0:
                        nc.sync.dma_start(
                            out=labR[:], in_=lab_all.to_broadcast((128, B))
                        )
        else:
            nc.sync.dma_start(out=labR[:], in_=lab_all.to_broadcast((128, B)))

        # ---- main loop ----
        ps_pool = ctx.enter_context(
            tc.tile_pool(name="ps", bufs=int(os.environ.get("PSBUFS", "3")), space="PSUM")
        )
        el_pool = ctx.enter_context(tc.tile_pool(name="el", bufs=4))
        st_pool = ctx.enter_context(tc.tile_pool(name="st", bufs=2))
        ex_pool = ctx.enter_context(tc.tile_pool(name="ex", bufs=2))

        UW = 1024                 # n-unit width (2 PSUM banks)
        NU = B // UW              # 8 n-units per row-chunk
        KORDER = [0, 1, 2, NU, 3, 4, 5, 6, 7]   # p-unit (NU) after its srcs
        units = [(mc, k) for mc in range(N_MC) for k in KORDER]
        T = len(units)
        psT = {}
        mnT = {}     # mc -> (mn, mn2) [128, W] bf16
        t2T = {}     # (mc, pair) -> staging [128, 4096] bf16
        tpT = {}     # mc -> p staging [128, W] bf16

        # per-col ns costs for the greedy balancer (calibrated vs CoreSim)
        C_PSUM_RD = 1.042   # DVE 1x from PSUM
        C_TT2X = 0.521
        C_TSP4X = 0.260
        C_ACT = 0.833
        C_POOL_TT = 0.84 * float(os.environ.get("POOLEFF", "1.0"))
        OH = 60.0           # per-instruction overhead (DVE/Pool)
        OHA = 160.0         # ACT per-op overhead (SBUF access latency)

        def win(mc):
            a = WOFF0 + mc * 128
            return a, a + W

        def s0(u):
            mc, k = units[u]
            lhs = fT[:, ROT + mc * 128 : ROT + (mc + 1) * 128]
            if k < NU:
                ps = ps_pool.tile([128, UW], F32, tag="ps")
                for h in range(UW // 512):
                    nsl = slice(k * UW + h * 512, k * UW + (h + 1) * 512)
                    nc.tensor.matmul(
                        ps[:, h * 512 : (h + 1) * 512], lhs, fT[:, nsl],
                        start=True, stop=True,
                    )
                psT[(mc, k)] = ps
                if k == 0:
                    # pre-charge ACT with this row-chunk's fixed exp cost so
                    # the greedy sees it before assigning flexible work
                    load["ACT"] += 2 * (C_ACT * PAIR + 187 + OHA) + (
                        C_ACT * W + 187 + OHA
                    )
                    wa, wb = win(mc)
                    mn = el_pool.tile([128, W], BF16, tag="mn", name="mn", bufs=2)
                    nc.vector.tensor_scalar(
                        mn[:], labR[:, wa:wb], labL[:, mc : mc + 1], MASKV,
                        op0=ALU.not_equal, op1=ALU.mult,
                    )
                    mn2 = el_pool.tile([128, W], BF16, tag="mn2", name="mn2", bufs=2)
                    nc.vector.tensor_scalar(
                        mn2[:], labR[:, wa:wb], labL[:, mc : mc + 1], -MASKV,
                        op0=ALU.is_equal, op1=ALU.mult,
                    )
                    load["DVE"] += 2 * (C_TSP4X * W + OH)
                    mnT[mc] = (mn, mn2)
            else:
                pass  # p-unit reuses n-unit PSUM tiles

        def emit_unmasked(ps, g0, a, b, t2, toff):
            """n-side r=max(u,-m); w2=r*r -> t2[:, toff:toff+(b-a)]."""
            wdt = b - a
            psl = ps[:, a - g0 : b - g0]
            tsl = t2[:, toff : toff + wdt]
            # read+max: DVE TSP (PSUM) or ACT Relu (PSUM); square:
            # DVE TT / ACT Square / Pool TT (SBUF only).
            plan = pick([
                ("A", {"DVE": (C_PSUM_RD + C_TT2X) * wdt + 2 * OH}),
                ("B", {"ACT": 2 * C_ACT * wdt + 2 * OHA}),
                ("C", {"DVE": C_PSUM_RD * wdt + OH, "ACT": C_ACT * wdt + OHA}),
                ("F", {"DVE": C_PSUM_RD * wdt + OH, "POOL": C_POOL_TT * wdt + OH}),
                ("G", {"ACT": C_ACT * wdt + OHA, "DVE": C_TSP4X * wdt + OH,
                       "POOL": C_POOL_TT * wdt + OH}),
            ])
            if plan == "B":
                v = el_pool.tile([128, wdt], BF16, tag="v", bufs=3)
                nc.scalar.activation(v[:], psl, AF.Relu, bias=M)
                nc.scalar.activation(tsl, v[:], AF.Square, bias=-M)
            elif plan == "G":
                v = el_pool.tile([128, wdt], BF16, tag="v", bufs=3)
                nc.scalar.activation(v[:], psl, AF.Relu, bias=M)
                a2 = el_pool.tile([128, wdt], BF16, tag="a2", bufs=3)
                nc.vector.tensor_scalar(
                    a2[:], v[:], -M, None, op0=ALU.add
                )
                nc.gpsimd.tensor_mul(tsl, a2[:], a2[:])
            else:
                r = el_pool.tile([128, wdt], BF16, tag="r", bufs=3)
                nc.vector.tensor_scalar(r[:], psl, -M, None, op0=ALU.max)
                if plan == "A":
                    nc.vector.tensor_mul(tsl, r[:], r[:])
                elif plan == "C":
                    nc.scalar.activation(tsl, r[:], AF.Square)
                else:  # F
                    nc.gpsimd.tensor_mul(tsl, r[:], r[:])

        def emit_masked(mc, ps, g0, a, b, t2, toff):
            """window n-side with mask fold."""
            wdt = b - a
            wa, _ = win(mc)
            psl = ps[:, a - g0 : b - g0]
            tsl = t2[:, toff : toff + wdt]
            _, mn2 = mnT[mc]
            msl = mn2[:, a - wa : b - wa]
            plan = pick([
                ("A", {"DVE": (C_PSUM_RD + 2 * C_TT2X) * wdt + 3 * OH}),
                ("B", {"ACT": 2 * C_ACT * wdt + 2 * OHA,
                       "DVE": C_TT2X * wdt + OH}),
                ("BP", {"ACT": 2 * C_ACT * wdt + 2 * OHA,
                        "POOL": C_POOL_TT * wdt + OH}),
                ("AP", {"DVE": (C_PSUM_RD + C_TT2X) * wdt + 2 * OH,
                        "POOL": C_POOL_TT * wdt + OH}),
                ("APP", {"DVE": C_PSUM_RD * wdt + OH,
                         "POOL": 2 * C_POOL_TT * wdt + 2 * OH}),
            ])
            if plan in ("B", "BP"):
                v = el_pool.tile([128, wdt], BF16, tag="v", bufs=3)
                nc.scalar.activation(v[:], psl, AF.Relu, bias=M)
                w2 = el_pool.tile([128, wdt], BF16, tag="w2", bufs=3)
                nc.scalar.activation(w2[:], v[:], AF.Square, bias=-M)
            else:
                r = el_pool.tile([128, wdt], BF16, tag="r", bufs=3)
                nc.vector.tensor_scalar(r[:], psl, -M, None, op0=ALU.max)
                w2 = el_pool.tile([128, wdt], BF16, tag="w2", bufs=3)
                if plan == "APP":
                    nc.gpsimd.tensor_mul(w2[:], r[:], r[:])
                else:
                    nc.vector.tensor_mul(w2[:], r[:], r[:])
            e = nc.gpsimd if plan in ("BP", "AP", "APP") else nc.vector
            e.tensor_add(tsl, w2[:], msl)

        def s1(u):
            mc, k = units[u]
            wa, wb = win(mc)
            if k < NU:
                ps = psT[(mc, k)]
                g0, g1 = k * UW, (k + 1) * UW
                pair = k // (PAIR // UW)
                if k % (PAIR // UW) == 0:
                    t2T[(mc, pair)] = el_pool.tile(
                        [128, PAIR], BF16, tag="t2", name="t2", bufs=2
                    )
                t2 = t2T[(mc, pair)]
                po = pair * PAIR
                ma, mb = max(g0, wa), min(g1, wb)
                segs = []
                if ma < mb:
                    if g0 < ma:
                        segs.append(("u", g0, ma))
                    segs.append(("m", ma, mb))
                    if mb < g1:
                        segs.append(("u", mb, g1))
                else:
                    segs.append(("u", g0, g1))
                for kind, a, b in segs:
                    if kind == "m":
                        emit_masked(mc, ps, g0, a, b, t2, a - po)
                    else:
                        emit_unmasked(ps, g0, a, b, t2, a - po)
            else:
                mn, _ = mnT[mc]
                w = el_pool.tile([128, W], BF16, tag="w", bufs=2)
                x = wa
                while x < wb:
                    ku = x // UW
                    y = min(wb, (ku + 1) * UW)
                    ps = psT[(mc, ku)]
                    psl = ps[:, x - ku * UW : y - ku * UW]
                    wsl = w[:, x - wa : y - wa]
                    pw = y - x
                    plan = pick([
                        ("ACT", {"ACT": C_ACT * pw + OHA}),
                        ("DVE", {"DVE": C_PSUM_RD * pw + OH,
                                 "POOL": C_POOL_TT * pw + OH}),
                    ])
                    if plan == "ACT":
                        nc.scalar.activation(
                            wsl, psl, AF.Square, bias=1.0, scale=-1.0
                        )
                    else:
                        a = el_pool.tile([128, pw], BF16, tag="a", bufs=2)
                        nc.vector.tensor_scalar(
                            a[:], psl, -1.0, 1.0,
                            op0=ALU.mult, op1=ALU.add,
                        )
                        nc.gpsimd.tensor_mul(wsl, a[:], a[:])
                    x = y
                tp = el_pool.tile([128, W], BF16, tag="tp", name="tp", bufs=2)
                fold = pick([
                    ("DVE", {"DVE": C_TT2X * W + OH}),
                    ("POOL", {"POOL": C_POOL_TT * W + OH}),
                ])
                if fold == "DVE":
                    nc.vector.tensor_sub(tp[:], w[:], mn[:])
                else:
                    nc.gpsimd.tensor_sub(tp[:], w[:], mn[:])
                tpT[mc] = tp

        def s2(u):
            mc, k = units[u]
            if k == NU:
                for kk in range(3):
                    psT.pop((mc, kk), None)
            elif k >= 3:
                psT.pop((mc, k), None)
            if k == 3 or k == NU - 1:
                pair = 0 if k == 3 else 1
                t2 = t2T.pop((mc, pair))
                ex = ex_pool.tile([128, PAIR], BF16, tag="ex")
                nc.scalar.activation(
                    ex[:, :PAIR], t2[:], AF.Exp, bias=-2.0, scale=S,
                    accum_out=stats[:, N_MC + 2 * mc + pair : N_MC + 2 * mc + pair + 1],
                )
            elif k == NU:
                tp = tpT.pop(mc)
                mnT.pop(mc)
                ex = ex_pool.tile([128, PAIR], BF16, tag="ex")
                nc.scalar.activation(
                    ex[:, :W], tp[:], AF.Exp, bias=-2.0, scale=S,
                    accum_out=stats[:, mc : mc + 1],
                )

        if loops == 0:
            nc.gpsimd.memset(stats[:], 0.0)
        SK = int(os.environ.get("SKEW", "2"))
        prep_load = dict(load)
        for rep in range(loops):
            psT.clear(); mnT.clear(); t2T.clear(); tpT.clear()
            load.update(prep_load)
            for c in range(T + SK):
                if c < T:
                    s0(c)
                if 1 <= c and c - 1 < T:
                    s1(c - 1)
                if SK <= c and c - SK < T:
                    s2(c - SK)
        nc.sync.dma_start(out=out, in_=stats[:])
        if os.environ.get("DEBUG_LOAD"):
            print("model load:", {k: round(v) for k, v in load.items()})
            print("plan hist:", plan_hist)
    nc.compile()
    return nc


def _make_in_maps(feats, labels, W=1536):
    """Sort by label, rotate per core, verify window containment."""
    feats = np.ascontiguousarray(np.asarray(feats), dtype=np.float32)
    labels = np.asarray(labels).reshape(-1).astype(np.int64)
    order = np.argsort(labels, kind="stable")
    sf = np.ascontiguousarray(feats[order])
    sl = labels[order]

    # group start/end in sorted coords
    uniq, starts = np.unique(sl, return_index=True)
    gs = {int(v): int(s) for v, s in zip(uniq, starts)}
    ge = {}
    for i, v in enumerate(uniq):
        ge[int(v)] = int(starts[i + 1]) if i + 1 < len(uniq) else B

    def fits(Wc):
        woff0 = ROT - (Wc - 128) // 2
        for rc in range(B // 128):
            c, mc = rc // N_MC, rc % N_MC
            lo = gs[int(sl[rc * 128])]
            hi = ge[int(sl[rc * 128 + 127])]
            rl = lo - (c * BL - ROT)
            rh = hi - (c * BL - ROT)
            wa = woff0 + mc * 128
            if rl < wa or rh > wa + Wc or wa < 0 or wa + Wc > PAIR:
                return False
        return True

    Wuse = None
    for Wc in (1280, 1408, W, 2048):
        if Wc > W and Wc != 2048:
            continue
        if fits(Wc):
            Wuse = Wc
            break
    assert Wuse is not None, "label groups too large for window"

    import ml_dtypes
    ident = np.eye(128, dtype=np.float32)
    in_maps = []
    for c in range(NCORES):
        rot = (np.arange(B) + c * BL - ROT) % B
        fc = np.ascontiguousarray(sf[rot])
        lc = sl[rot]
        in_maps.append({
            "feats": fc,
            "lab_all": lc.astype(ml_dtypes.bfloat16).reshape(1, -1),
            "lab_loc": np.ascontiguousarray(
                lc[ROT : ROT + BL].reshape(N_MC, 128).T.astype(np.float32)
            ),
            "ident": ident,
        })
    return in_maps, Wuse


def kernel(feats, labels):
    global LAST_RESULTS
    in_maps, Wuse = _make_in_maps(feats, labels)
    key = (1, 1, Wuse)
    if key not in _NC_CACHE:
        _NC_CACHE[key] = _build_nc(loops=1, prep=1, W=Wuse)
    nc = _NC_CACHE[key]

    res = run_bass_kernel_spmd(
        nc, in_maps, list(range(NCORES)),
        trace=bool(os.environ.get("KERNEL_TRACE")),
    )
    LAST_RESULTS = res

    P_parts, N_parts = [], []
    for c in range(NCORES):
        st = res.results[c]["out"]            # [128, 24]
        P_parts.append(st[:, :N_MC].T.reshape(-1))
        N_parts.append(
            (st[:, N_MC::2] + st[:, N_MC + 1 :: 2]).T.reshape(-1)
        )
    P = np.concatenate(P_parts) - np.float32(np.exp(-2.0))
    N = np.concatenate(N_parts)
    loss_rows = np.log1p(P.astype(np.float32) * N.astype(np.float32))
    return np.float32(np.mean(loss_rows))


# revision 26
# speedup vs baseline: 2.6163x; 1.5146x over previous
"""CircleLoss kernel for Trainium2 (Bass/Tile), SPMD across 8 NeuronCores.

v2: label-sorted + per-core rotated layout.

Math (s=32, m=0.25, B=8192, D=128), u = cosine sim:
    p-side: expo_p = s*(1-u)^2 - 2           (relu never clips: u <= 1)
    n-side: expo_n = s*max(u,-m)^2 - 2       (exact, incl. clamp at -m)
    loss = mean(log1p((P - e^-2) * N)), P/N per-row sums of exp(expo).

Layout trick: rows are sorted by label on the host. Each core's input is
the sorted row set ROTATED so its own 1024 rows sit at columns
[960, 1984) of its local fT. All same-label columns for local row-chunk
mc then provably lie in a fixed window [mc*128+WOFF0, +W) (host verifies
group sizes; W=1536 or 2048). The SPMD program is identical across
cores; all data-dependence lives in the inputs (rotated feats + labels).

Per row-chunk mc (128 rows):
  - n-side: 4 PSUM groups of 2048 cols over all 8192 cols. Columns
    outside the window are guaranteed different-label -> no mask:
    r = max(u,-m) (Pool/DVE/ACT), w2 = r*r into bf16 staging. Window
    columns get a mask mn2 = -4*(lab_i==lab_j) folded additively
    (exp arg shifts by -128 -> 0). Two Exp+accum per row [128,4096].
  - p-side: 1 extra PSUM tile [128, W] over the window columns only:
    w = (1-u)^2 via ACT Square from PSUM, tp = w - mn (mn=4*(neq)),
    one Exp+accum [128, W]. Diagonal contributes exp(-2), subtracted
    on the host.

Output per core: [128, 24] f32: cols 0..7 = p row-sums per mc,
cols 8+2mc, 9+2mc = n partial row-sums. Host: mean(log1p((P-e^-2)*N)).
"""

import os
import numpy as np
from contextlib import ExitStack

import concourse.bass as bass
import concourse.bacc as bacc
import concourse.tile as tile
import concourse.mybir as mybir
from concourse.bass_utils import run_bass_kernel_spmd

B, D, NCORES = 8192, 128, 8
BL = B // NCORES          # 1024 rows per core
N_MC = BL // 128          # 8 row-chunks per core
S, M = 32.0, 0.25
MASKV = 4.0               # exp-arg shift: -S*4 = -128
ROT = 960                 # local rows sit at rotated cols [960, 1984)
GW = 2048                 # PSUM group width (4 banks)
PAIR = 4096               # n-exp width (2 groups)
F32 = mybir.dt.float32
F32R = mybir.dt.float32r
BF16 = mybir.dt.bfloat16
AF = mybir.ActivationFunctionType
ALU = mybir.AluOpType

_NC_CACHE = {}
LAST_RESULTS = None


def _register_consts(nc, values):
    for v in values:
        key = (F32, float(v))
        if key in nc.const_aps.aps:
            continue
        t = nc.alloc_sbuf_tensor(f"const-f32-{v}", [128, 1], F32)
        nc.gpsimd.memset(t.ap(), float(v))
        nc.const_aps.aps[key] = t.ap()
    nc.all_engine_barrier()


def _build_nc(loops=1, prep=1, W=1536):
    WOFF0 = ROT - (W - 128) // 2      # 256 for W=1536, 0 for W=2048
    nc = bacc.Bacc(
        "TRN2", target_bir_lowering=False, debug=False, num_devices=NCORES
    )
    _register_consts(nc, [-2.0, 0.25, -0.25])
    feats = nc.dram_tensor("feats", [B, D], F32, kind="ExternalInput").ap()
    lab_all = nc.dram_tensor("lab_all", [1, B], BF16, kind="ExternalInput").ap()
    lab_loc = nc.dram_tensor("lab_loc", [128, N_MC], F32, kind="ExternalInput").ap()
    ident = nc.dram_tensor("ident", [128, 128], F32, kind="ExternalInput").ap()
    out = nc.dram_tensor("out", [128, 3 * N_MC], F32, kind="ExternalOutput").ap()

    # engine-load tracker for greedy assignment (ns, coarse cost model)
    load = {"ACT": 0.0, "DVE": 0.0, "POOL": 0.0}

    plan_hist = {}

    def pick(variants):
        """variants: (key, {eng: cost}) list; min-max with full-vector
        lexicographic tie-break so ties don't fall to declaration order."""
        best, bestv = None, None
        for key, costs in variants:
            vec = sorted(
                (load[e] + costs.get(e, 0.0) for e in load), reverse=True
            )
            if bestv is None or vec < bestv:
                best, bestv = key, vec
        _, costs = next(v for v in variants if v[0] == best)
        for e, c in costs.items():
            load[e] += c
        plan_hist[best] = plan_hist.get(best, 0) + 1
        return best

    with tile.TileContext(nc) as tc, ExitStack() as ctx:
        persist = ctx.enter_context(tc.tile_pool(name="persist", bufs=1))
        ft_pool = ctx.enter_context(tc.tile_pool(name="ft", bufs=3))
        sm_pool = ctx.enter_context(tc.tile_pool(name="sm", bufs=4))

        fT = persist.tile([128, B], F32R, name="fT")
        labR = persist.tile([128, B], BF16, name="labR")
        labL = persist.tile([128, N_MC], F32, name="labL")
        idn = persist.tile([128, 128], F32, name="idn")
        stats = persist.tile([128, 3 * N_MC], F32, name="stats")

        nc.sync.dma_start(out=idn[:], in_=ident)
        nc.sync.dma_start(out=labL[:], in_=lab_loc)

        def norm_transpose_batch(tp_pool, b):
            """Normalize+transpose rows [b*1024, (b+1)*1024) into fT cols."""
            load["POOL"] += 860                    # sq
            load["DVE"] += 1127 + 100 + 4 * 127 + 4 * 258
            load["ACT"] += 4 * 292                 # fnorm act-copies
            load["ACT"] += 192 + 4 * 292          # sqrt + copies
            fb = ft_pool.tile([128, 8, 128], F32, tag="fb")
            for t in range(8):
                r0 = b * 1024 + t * 128
                nc.sync.dma_start(out=fb[:, t, :], in_=feats[r0 : r0 + 128, :])
            sq = ft_pool.tile([128, 8, 128], F32, tag="sq")
            pe_ = float(os.environ.get("POOLEFF", "1.0"))
            if pe_ < 1.5 or (pe_ < 10 and b % 2 == 0):
                nc.gpsimd.tensor_mul(sq[:], fb[:], fb[:])
            else:
                nc.scalar.activation(sq[:], fb[:], AF.Square)
            ssq = sm_pool.tile([128, 8, 1], F32, tag="ssq")
            nc.vector.tensor_reduce(
                ssq[:], sq[:], axis=mybir.AxisListType.X, op=ALU.add
            )
            nrm = sm_pool.tile([128, 8, 1], F32, tag="nrm")
            nc.scalar.activation(nrm[:], ssq[:], AF.Sqrt)
            inv = sm_pool.tile([128, 8, 1], F32, tag="inv")
            nc.vector.reciprocal(inv[:], nrm[:])
            for t in range(8):
                fn = ft_pool.tile([128, 128], F32, tag="fn", bufs=4)
                if t % 2 == 0:
                    nc.vector.tensor_scalar_mul(fn[:], fb[:, t, :], inv[:, t, :])
                else:
                    nc.scalar.activation(
                        fn[:], fb[:, t, :], AF.Copy, scale=inv[:, t, :]
                    )
                pt = tp_pool.tile([128, 128], F32, tag="pt")
                nc.tensor.transpose(pt[:], fn[:], idn[:])
                c0 = (b * 8 + t) * 128
                if t % 2 == 0:
                    nc.vector.tensor_copy(fT[:, c0 : c0 + 128], pt[:])
                else:
                    nc.scalar.copy(fT[:, c0 : c0 + 128], pt[:])

        tp_pool = ctx.enter_context(tc.tile_pool(name="tp", bufs=2, space="PSUM"))
        if prep:
            for _rep in range(prep):
                for b in range(8):
                    norm_transpose_batch(tp_pool, b)
                    if b == 0 and _rep == 0:
                        nc.sync.dma_start(
                            out=labR[:], in_=lab_all.to_broadcast((128, B))
                        )
        else:
            nc.sync.dma_start(out=labR[:], in_=lab_all.to_broadcast((128, B)))

        # ---- main loop ----
        ps_pool = ctx.enter_context(
            tc.tile_pool(name="ps", bufs=int(os.environ.get("PSBUFS", "3")), space="PSUM")
        )
        el_pool = ctx.enter_context(tc.tile_pool(name="el", bufs=4))
        st_pool = ctx.enter_context(tc.tile_pool(name="st", bufs=2))
        ex_pool = ctx.enter_context(tc.tile_pool(name="ex", bufs=2))

        UW = 1024                 # n-unit width (2 PSUM banks)
        NU = B // UW              # 8 n-units per row-chunk
        KORDER = [0, 1, 2, NU, 3, 4, 5, 6, 7]   # p-unit (NU) after its srcs
        units = [(mc, k) for mc in range(N_MC) for k in KORDER]
        T = len(units)
        psT = {}
        mnT = {}     # mc -> (mn, mn2) [128, W] bf16
        t2T = {}     # (mc, pair) -> staging [128, 4096] bf16
        tpT = {}     # mc -> p staging [128, W] bf16

        # per-col ns costs for the greedy balancer (calibrated vs CoreSim)
        C_PSUM_RD = 1.042   # DVE 1x from PSUM
        C_TT2X = 0.521
        C_TSP4X = 0.260
        C_ACT = 0.833
        C_POOL_TT = 0.84 * float(os.environ.get("POOLEFF", "1.0"))
        OH = 60.0           # per-instruction overhead (DVE/Pool)
        OHA = 160.0         # ACT per-op overhead (SBUF access latency)

        def win(mc):
            a = WOFF0 + mc * 128
            return a, a + W

        def s0(u):
            mc, k = units[u]
            lhs = fT[:, ROT + mc * 128 : ROT + (mc + 1) * 128]
            if k < NU:
                ps = ps_pool.tile([128, UW], F32, tag="ps")
                for h in range(UW // 512):
                    nsl = slice(k * UW + h * 512, k * UW + (h + 1) * 512)
                    nc.tensor.matmul(
                        ps[:, h * 512 : (h + 1) * 512], lhs, fT[:, nsl],
                        start=True, stop=True,
                    )
                psT[(mc, k)] = ps
                if k == 0:
                    # pre-charge ACT with this row-chunk's fixed exp cost so
                    # the greedy sees it before assigning flexible work
                    load["ACT"] += 2 * (C_ACT * PAIR + 187 + OHA) + (
                        C_ACT * W + 187 + OHA
                    )
                    wa, wb = win(mc)
                    mn = el_pool.tile([128, W], BF16, tag="mn", name="mn", bufs=2)
                    nc.vector.tensor_scalar(
                        mn[:], labR[:, wa:wb], labL[:, mc : mc + 1], MASKV,
                        op0=ALU.not_equal, op1=ALU.mult,
                    )
                    mn2 = el_pool.tile([128, W], BF16, tag="mn2", name="mn2", bufs=2)
                    nc.vector.tensor_scalar(
                        mn2[:], labR[:, wa:wb], labL[:, mc : mc + 1], -MASKV,
                        op0=ALU.is_equal, op1=ALU.mult,
                    )
                    load["DVE"] += 2 * (C_TSP4X * W + OH)
                    mnT[mc] = (mn, mn2)
            else:
                pass  # p-unit reuses n-unit PSUM tiles

        def emit_unmasked(ps, g0, a, b, t2, toff):
            """n-side r=max(u,-m); w2=r*r -> t2[:, toff:toff+(b-a)]."""
            wdt = b - a
            psl = ps[:, a - g0 : b - g0]
            tsl = t2[:, toff : toff + wdt]
            # read+max: DVE TSP (PSUM) or ACT Relu (PSUM); square:
            # DVE TT / ACT Square / Pool TT (SBUF only).
            plan = pick([
                ("A", {"DVE": (C_PSUM_RD + C_TT2X) * wdt + 2 * OH}),
                ("B", {"ACT": 2 * C_ACT * wdt + 2 * OHA}),
                ("C", {"DVE": C_PSUM_RD * wdt + OH, "ACT": C_ACT * wdt + OHA}),
                ("F", {"DVE": C_PSUM_RD * wdt + OH, "POOL": C_POOL_TT * wdt + OH}),
                ("G", {"ACT": C_ACT * wdt + OHA, "DVE": C_TSP4X * wdt + OH,
                       "POOL": C_POOL_TT * wdt + OH}),
            ])
            if plan == "B":
                v = el_pool.tile([128, wdt], BF16, tag="v", bufs=3)
                nc.scalar.activation(v[:], psl, AF.Relu, bias=M)
                nc.scalar.activation(tsl, v[:], AF.Square, bias=-M)
            elif plan == "G":
                v = el_pool.tile([128, wdt], BF16, tag="v", bufs=3)
                nc.scalar.activation(v[:], psl, AF.Relu, bias=M)
                a2 = el_pool.tile([128, wdt], BF16, tag="a2", bufs=3)
                nc.vector.tensor_scalar(
                    a2[:], v[:], -M, None, op0=ALU.add
                )
                nc.gpsimd.tensor_mul(tsl, a2[:], a2[:])
            else:
                r = el_pool.tile([128, wdt], BF16, tag="r", bufs=3)
                nc.vector.tensor_scalar(r[:], psl, -M, None, op0=ALU.max)
                if plan == "A":
                    nc.vector.tensor_mul(tsl, r[:], r[:])
                elif plan == "C":
                    nc.scalar.activation(tsl, r[:], AF.Square)
                else:  # F
                    nc.gpsimd.tensor_mul(tsl, r[:], r[:])

        def emit_masked(mc, ps, g0, a, b, t2, toff):
            """window n-side with mask fold."""
            wdt = b - a
            wa, _ = win(mc)
            psl = ps[:, a - g0 : b - g0]
            tsl = t2[:, toff : toff + wdt]
            _, mn2 = mnT[mc]
            msl = mn2[:, a - wa : b - wa]
            plan = pick([
                ("A", {"DVE": (C_PSUM_RD + 2 * C_TT2X) * wdt + 3 * OH}),
                ("B", {"ACT": 2 * C_ACT * wdt + 2 * OHA,
                       "DVE": C_TT2X * wdt + OH}),
                ("BP", {"ACT": 2 * C_ACT * wdt + 2 * OHA,
                        "POOL": C_POOL_TT * wdt + OH}),
                ("AP", {"DVE": (C_PSUM_RD + C_TT2X) * wdt + 2 * OH,
                        "POOL": C_POOL_TT * wdt + OH}),
                ("APP", {"DVE": C_PSUM_RD * wdt + OH,
                         "POOL": 2 * C_POOL_TT * wdt + 2 * OH}),
            ])
            if plan in ("B", "BP"):
                v = el_pool.tile([128, wdt], BF16, tag="v", bufs=3)
                nc.scalar.activation(v[:], psl, AF.Relu, bias=M)
                w2 = el_pool.tile([128, wdt], BF16, tag="w2", bufs=3)
                nc.scalar.activation(w2[:], v[:], AF.Square, bias=-M)
            else:
                r = el_pool.tile([128, wdt], BF16, tag="r", bufs=3)
                nc.vector.tensor_scalar(r[:], psl, -M, None, op0=ALU.max)
                w2 = el_pool.tile([128, wdt], BF16, tag="w2", bufs=3)
                if plan == "APP":
                    nc.gpsimd.tensor_mul(w2[:], r[:], r[:])
                else:
                    nc.vector.tensor_mul(w2[:], r[:], r[:])
            e = nc.gpsimd if plan in ("BP", "AP", "APP") else nc.vector
            e.tensor_add(tsl, w2[:], msl)

        def s1(u):
            mc, k = units[u]
            wa, wb = win(mc)
            if k < NU:
                ps = psT[(mc, k)]
                g0, g1 = k * UW, (k + 1) * UW
                pair = k // (PAIR // UW)
                if k % (PAIR // UW) == 0:
                    t2T[(mc, pair)] = el_pool.tile(
                        [128, PAIR], BF16, tag="t2", name="t2", bufs=2
                    )
                t2 = t2T[(mc, pair)]
                po = pair * PAIR
                ma, mb = max(g0, wa), min(g1, wb)
                segs = []
                if ma < mb:
                    if g0 < ma:
                        segs.append(("u", g0, ma))
                    segs.append(("m", ma, mb))
                    if mb < g1:
                        segs.append(("u", mb, g1))
                else:
                    segs.append(("u", g0, g1))
                for kind, a, b in segs:
                    if kind == "m":
                        emit_masked(mc, ps, g0, a, b, t2, a - po)
                    else:
                        emit_unmasked(ps, g0, a, b, t2, a - po)
            else:
                mn, _ = mnT[mc]
                w = el_pool.tile([128, W], BF16, tag="w", bufs=2)
                x = wa
                while x < wb:
                    ku = x // UW
                    y = min(wb, (ku + 1) * UW)
                    ps = psT[(mc, ku)]
                    psl = ps[:, x - ku * UW : y - ku * UW]
                    wsl = w[:, x - wa : y - wa]
                    pw = y - x
                    plan = pick([
                        ("ACT", {"ACT": C_ACT * pw + OHA}),
                        ("DVE", {"DVE": C_PSUM_RD * pw + OH,
                                 "POOL": C_POOL_TT * pw + OH}),
                    ])
                    if plan == "ACT":
                        nc.scalar.activation(
                            wsl, psl, AF.Square, bias=1.0, scale=-1.0
                        )
                    else:
                        a = el_pool.tile([128, pw], BF16, tag="a", bufs=2)
                        nc.vector.tensor_scalar(
                            a[:], psl, -1.0, 1.0,
                            op0=ALU.mult, op1=ALU.add,
                        )
                        nc.gpsimd.tensor_mul(wsl, a[:], a[:])
                    x = y
                tp = el_pool.tile([128, W], BF16, tag="tp", name="tp", bufs=2)
                fold = pick([
                    ("DVE", {"DVE": C_TT2X * W + OH}),
                    ("POOL", {"POOL": C_POOL_TT * W + OH}),
                ])
                if fold == "DVE":
                    nc.vector.tensor_sub(tp[:], w[:], mn[:])
                else:
                    nc.gpsimd.tensor_sub(tp[:], w[:], mn[:])
                tpT[mc] = tp

        def s2(u):
            mc, k = units[u]
            if k == NU:
                for kk in range(3):
                    psT.pop((mc, kk), None)
            elif k >= 3:
                psT.pop((mc, k), None)
            if k == 3 or k == NU - 1:
                pair = 0 if k == 3 else 1
                t2 = t2T.pop((mc, pair))
                ex = ex_pool.tile([128, PAIR], BF16, tag="ex")
                nc.scalar.activation(
                    ex[:, :PAIR], t2[:], AF.Exp, bias=-2.0, scale=S,
                    accum_out=stats[:, N_MC + 2 * mc + pair : N_MC + 2 * mc + pair + 1],
                )
            elif k == NU:
                tp = tpT.pop(mc)
                mnT.pop(mc)
                ex = ex_pool.tile([128, PAIR], BF16, tag="ex")
                nc.scalar.activation(
                    ex[:, :W], tp[:], AF.Exp, bias=-2.0, scale=S,
                    accum_out=stats[:, mc : mc + 1],
                )

        if loops == 0:
            nc.gpsimd.memset(stats[:], 0.0)
        SK = int(os.environ.get("SKEW", "2"))
        prep_load = dict(load)
        for rep in range(loops):
            psT.clear(); mnT.clear(); t2T.clear(); tpT.clear()
            load.update(prep_load)
            for c in range(T + SK):
                if c < T:
                    s0(c)
                if 1 <= c and c - 1 < T:
                    s1(c - 1)
                if SK <= c and c - SK < T:
                    s2(c - SK)
        nc.sync.dma_start(out=out, in_=stats[:])
        if os.environ.get("DEBUG_LOAD"):
            print("model load:", {k: round(v) for k, v in load.items()})
            print("plan hist:", plan_hist)
    nc.compile()
    return nc


def _make_in_maps(feats, labels, W=1536):
    """Sort by label, rotate per core, verify window containment."""
    feats = np.ascontiguousarray(np.asarray(feats), dtype=np.float32)
    labels = np.asarray(labels).reshape(-1).astype(np.int64)
    order = np.argsort(labels, kind="stable")
    sf = np.ascontiguousarray(feats[order])
    sl = labels[order]

    # group start/end in sorted coords
    uniq, starts = np.unique(sl, return_index=True)
    gs = {int(v): int(s) for v, s in zip(uniq, starts)}
    ge = {}
    for i, v in enumerate(uniq):
        ge[int(v)] = int(starts[i + 1]) if i + 1 < len(uniq) else B

    def fits(Wc):
        woff0 = ROT - (Wc - 128) // 2
        for rc in range(B // 128):
            c, mc = rc // N_MC, rc % N_MC
            lo = gs[int(sl[rc * 128])]
            hi = ge[int(sl[rc * 128 + 127])]
            rl = lo - (c * BL - ROT)
            rh = hi - (c * BL - ROT)
            wa = woff0 + mc * 128
            if rl < wa or rh > wa + Wc or wa < 0 or wa + Wc > PAIR:
                return False
        return True

    Wuse = None
    for Wc in (1280, 1408, W, 2048):
        if Wc > W and Wc != 2048:
            continue
        if fits(Wc):
            Wuse = Wc
            break
    assert Wuse is not None, "label groups too large for window"

    import ml_dtypes
    ident = np.eye(128, dtype=np.float32)
    in_maps = []
    for c in range(NCORES):
        rot = (np.arange(B) + c * BL - ROT) % B
        fc = np.ascontiguousarray(sf[rot])
        lc = sl[rot]
        in_maps.append({
            "feats": fc,
            "lab_all": lc.astype(ml_dtypes.bfloat16).reshape(1, -1),
            "lab_loc": np.ascontiguousarray(
                lc[ROT : ROT + BL].reshape(N_MC, 128).T.astype(np.float32)
            ),
            "ident": ident,
        })
    return in_maps, Wuse


def kernel(feats, labels):
    global LAST_RESULTS
    in_maps, Wuse = _make_in_maps(feats, labels)
    key = (1, 1, Wuse)
    if key not in _NC_CACHE:
        _NC_CACHE[key] = _build_nc(loops=1, prep=1, W=Wuse)
    nc = _NC_CACHE[key]

    res = run_bass_kernel_spmd(
        nc, in_maps, list(range(NCORES)),
        trace=bool(os.environ.get("KERNEL_TRACE")),
    )
    LAST_RESULTS = res

    P_parts, N_parts = [], []
    for c in range(NCORES):
        st = res.results[c]["out"]            # [128, 24]
        P_parts.append(st[:, :N_MC].T.reshape(-1))
        N_parts.append(
            (st[:, N_MC::2] + st[:, N_MC + 1 :: 2]).T.reshape(-1)
        )
    P = np.concatenate(P_parts) - np.float32(np.exp(-2.0))
    N = np.concatenate(N_parts)
    loss_rows = np.log1p(P.astype(np.float32) * N.astype(np.float32))
    return np.float32(np.mean(loss_rows))
